# revision 26
# baseline (speedup 1.0000x reference)
"""AdaConv2D (instance-norm -> grouped 3x3 conv -> grouped 1x1 conv -> bias) on 8 TRN2 cores.

v2 strategy (pure data parallel, 1 sample/core, no collectives).  Key change
vs v1 (195.5us): the instance-norm is FOLDED INTO THE WEIGHTS instead of
materialized as a normalize pass over x:

  out = conv(xn) + b,  xn = (x - mean)*rstd
      = conv_{w*rstd}(x_raw) + (b - sum_taps w*rstd*mean)

  - Device computes mean/rstd per channel (DVE bn_stats one pass + tiny ACT
    Ln/Exp chain), scales the chunk's weights by rstd (one ACT op, 576
    elems/partition), and runs the conv directly on RAW bf16 x.
  - H-edges: the 1-row top/bottom halo is filled with the per-channel MEAN
    (not zero): out-of-image taps then contribute w*mean which exactly
    cancels against the folded bias correction.
  - Bias correction corr_j = sum_{i,9 taps} w_s[i,j]*mean_i is computed on
    the PE itself: 36 N=1 matvec accumulations (4 quadrants x 9 taps) into a
    PSUM tile, ~1us/chunk.  bc = bias - corr (gpsimd).
  - W-edges keep the shrunk-AP taps; their (sub-1e-3) bias-corr discrepancy
    is left uncorrected - measured total rel err 1.2e-3 vs budget 2e-2.

  This removes the entire normalize pass (was ~17us/chunk of gpsimd/DVE) and
  the load->stats->chain->normalize critical path that held the first matmul
  until 41us and caused 16us of mid-kernel PE stalls.

  Engine layout per 128-channel chunk (4 chunks/sample):
  - Sync (HWDGE): all DMA. in: 4x1MB slices, two chunks ahead; out: 1MB
    chunks per spatial half (quarters on the last chunk).
  - DVE: bn_stats x32 (one pass over x as slices land) + bn_aggr.
  - ACT: rstd chain (Ln/Exp), mean->bf16, weight scaling, and ALL PSUM
    eviction (activation Identity with per-partition bias AP, psum f32 ->
    bf16 staging; ACT is the fastest eviction engine at ~0.71ns/elem).
  - GpSimd: halo fills (2 broadcast ops) + bc = bias - corr.  Nothing bulk.
  - TensorE: conv as 4 concurrent 64x64 tile_position matmuls (2 channel
    sub-chunk PAIRS x 2 spatial halves), 9 taps accumulated in PSUM.
    TAP-OUTER over windows of 2 spans: per tap one pair of lhsT loads serves
    8 matmuls (2 spans x 4 quadrants), halving exposure to the 64-col
    LDWEIGHTS serialization that bounds v1 (~212ns/tap of weight load vs
    ~107ns of bf16 moving-operand streaming).
  - PSUM: single 8-buf pool; each window holds 4 banks, corr rides the same
    rotation as a full-bank tile once per chunk.
"""
import os
import sys
import numpy as np
import ml_dtypes

if "/opt/trn_rl_repo" not in sys.path:
    sys.path.insert(0, "/opt/trn_rl_repo")

B, C, H, W = 8, 512, 128, 128
HW = H * W            # 16384
NCH = 4               # 128-channel chunks per sample
NTAP = 9
ROWS_PAD = H + 2      # 130 rows of 128 in padded SBUF layout
PADF = ROWS_PAD * W   # 16640 elems per partition
# taps ordered so the first three are dw=0 (full-width writes -> correct PSUM init)
TAPS = [(0, 1), (1, 1), (2, 1), (0, 0), (1, 0), (2, 0), (0, 2), (1, 2), (2, 2)]

_CACHE = {}


def _build_program():
    import concourse.bass as bass
    import concourse.tile as tile
    from concourse import bacc, mybir

    f32 = mybir.dt.float32
    bf16 = mybir.dt.bfloat16
    COPY = mybir.ActivationFunctionType.Copy
    IDENT = mybir.ActivationFunctionType.Identity
    SQRT = mybir.ActivationFunctionType.Sqrt
    EPS = 1e-7
    nc = bacc.Bacc("TRN2", target_bir_lowering=False, debug=False,
                   enable_asserts=False, num_devices=8)

    x_d = nc.dram_tensor("x", [C, HW], bf16, kind="ExternalInput")
    w_d = nc.dram_tensor("w", [128, NCH * NTAP * 64], bf16, kind="ExternalInput")
    b_d = nc.dram_tensor("bias", [128, 8], f32, kind="ExternalInput")
    out_d = nc.dram_tensor("out", [C, HW], bf16, kind="ExternalOutput")

    # store view: [cc, Ch(spatial half), hh(drain half), p, R, e(4096)]
    out_v = out_d[:].rearrange("(a R p) (Ch hh e) -> a Ch hh p R e", a=NCH, R=2,
                               p=64, Ch=2, hh=2, e=4096)

    with tile.TileContext(nc) as tc:
        with (
            tc.tile_pool(name="xpool", bufs=3) as xpool,
            tc.tile_pool(name="wpool", bufs=1) as wpool,
            tc.tile_pool(name="spool", bufs=3) as spool,
            tc.tile_pool(name="opool", bufs=2) as opool,
            tc.tile_pool(name="psum", bufs=8, space=bass.MemorySpace.PSUM) as pspool,
        ):
            w_sb = wpool.tile([128, NCH * NTAP * 64], bf16)
            nc.sync.dma_start(w_sb[:], w_d[:])
            bias_sb = wpool.tile([128, 8], f32)
            nc.sync.dma_start(bias_sb[:], b_d[:])
            ones_sb = wpool.tile([128, W], bf16)
            nc.gpsimd.memset(ones_sb[:], 1.0)
            trash0 = wpool.tile([128, 4096], bf16)
            trash1 = wpool.tile([128, 4096], bf16)

            st = {}  # per-chunk small tiles

            def emit_load(cc):
                xt = xpool.tile([128, PADF], bf16, tag="xt", name=f"xt{cc}")
                st[cc] = {"xt": xt}
                for k in range(4):
                    nc.sync.dma_start(xt[:, W + k * 4096: W + (k + 1) * 4096],
                                      x_d[cc * 128:(cc + 1) * 128,
                                          k * 4096:(k + 1) * 4096])

            def emit_stats_dve(cc, k, pos=None):
                # DVE bn_stats over slice k (8 blocks of 512), single pass;
                # pos = 8-block position within the 24-block stats6 tile
                s = st[cc]
                if pos is None:
                    pos = k
                if pos == 0:
                    s["stats6"] = spool.tile([128, 24 * 6], f32, tag="stats",
                                             name=f"st{cc}")
                xt = s["xt"]
                for jj in range(8):
                    j = 8 * k + jj
                    o = (8 * pos + jj) * 6
                    nc.vector.bn_stats(s["stats6"][:, o:o + 6],
                                       xt[:, W + j * 512: W + (j + 1) * 512])

            def emit_stats_act(cc, k=3):
                # ACT: sum + sumsq of slice k via Copy/Square accumulators
                s = st[cc]
                acc = spool.tile([128, 2], f32, tag="acc", name=f"ac{cc}")
                sl = s["xt"][:, W + k * 4096: W + (k + 1) * 4096]
                nc.scalar.activation(trash0[:], sl, COPY,
                                     accum_out=acc[:, 0:1])
                nc.scalar.activation(trash1[:], sl,
                                     mybir.ActivationFunctionType.Square,
                                     accum_out=acc[:, 1:2])
                s["acc"] = acc

            def emit_aggr(cc):
                s = st[cc]
                mv = spool.tile([128, 2], f32, tag="mv", name=f"mv{cc}")
                nc.vector.bn_aggr(mv[:], s["stats6"][:].rearrange(
                    "p (h s) -> p h s", s=6))
                s["mv24"] = mv

            def emit_mix(cc):
                # gpsimd: merge DVE 24-block stats (3/4 of chunk) with the
                # ACT accumulators (1/4): mean, var, all [128,1] f32 ops
                s = st[cc]
                g = nc.gpsimd
                ADD = mybir.AluOpType.add
                MUL = mybir.AluOpType.mult
                mv = s["mv24"]
                acc = s["acc"]
                t0 = spool.tile([128, 2], f32, tag="t0", name=f"t0{cc}")
                g.tensor_scalar_mul(t0[:], acc[:], 1.0 / HW)
                t1 = spool.tile([128, 2], f32, tag="t1", name=f"t1{cc}")
                g.tensor_scalar_mul(t1[:], mv[:], 0.75)  # [.75 m24, .75 v24]
                mean = spool.tile([128, 1], f32, tag="mean", name=f"me{cc}")
                g.tensor_add(mean[:], t1[:, 0:1], t0[:, 0:1])
                m2a = spool.tile([128, 1], f32, tag="m2a", name=f"ma{cc}")
                g.tensor_mul(m2a[:], t1[:, 0:1], mv[:, 0:1])  # .75 m24^2
                e2a = spool.tile([128, 1], f32, tag="e2a", name=f"ea{cc}")
                g.tensor_add(e2a[:], t1[:, 1:2], m2a[:])
                ex2 = spool.tile([128, 1], f32, tag="ex2", name=f"ex{cc}")
                g.tensor_add(ex2[:], e2a[:], t0[:, 1:2])
                m2 = spool.tile([128, 1], f32, tag="m2", name=f"m2{cc}")
                g.tensor_mul(m2[:], mean[:], mean[:])
                var = spool.tile([128, 1], f32, tag="var", name=f"va{cc}")
                g.tensor_sub(var[:], ex2[:], m2[:])
                s["mean"] = mean
                s["var"] = var

            def emit_chain(cc):
                # rstd = 1/(sqrt(var*N/(N-1)) + eps).  Sqrt/Copy/Identity/
                # Square share one ACT table set -> no table reloads (Ln/Exp
                # forced a 1.3us table swap per use).
                s = st[cc]
                lg = spool.tile([128, 1], f32, tag="lg", name=f"lg{cc}")
                nc.scalar.activation(lg[:], s["var"][:],
                                     mybir.ActivationFunctionType.Ln,
                                     scale=float(HW) / float(HW - 1))
                rstd = spool.tile([128, 1], f32, tag="rstd", name=f"rs{cc}")
                nc.scalar.activation(rstd[:], lg[:],
                                     mybir.ActivationFunctionType.Exp,
                                     scale=-0.5)
                mb = spool.tile([128, 1], bf16, tag="mb", name=f"mb{cc}")
                nc.scalar.activation(mb[:], s["mean"][:], COPY)
                s["rstd"] = rstd
                s["mb"] = mb

            def emit_wscale(cc, on_gp=False):
                # scale this chunk's weights by rstd (per input channel row).
                # chunk 0 runs on the idle gpsimd queue so it is not stuck
                # behind chunk 1's bn_stats on DVE in the prologue.
                s = st[cc]
                wsc = spool.tile([128, NTAP * 64], bf16, tag="wsc",
                                 name=f"ws{cc}")
                src = w_sb[:, cc * NTAP * 64:(cc + 1) * NTAP * 64]
                eng = nc.gpsimd if on_gp else nc.vector
                eng.tensor_scalar_mul(wsc[:], src, s["rstd"][:, 0:1])
                s["wsc"] = wsc

            def emit_halo(cc):
                # top/bottom halo row := per-channel mean (bf16) so that
                # out-of-image dh taps cancel against the folded bias corr.
                # scalar operand must be f32; the bf16 output rounds to the
                # same value as the mb tile used by the corr matvec.
                s = st[cc]
                xt = s["xt"]
                mf = s["mean"][:, 0:1]
                nc.gpsimd.tensor_scalar_mul(xt[:, 0:W], ones_sb[:], mf)
                nc.gpsimd.tensor_scalar_mul(xt[:, PADF - W:PADF], ones_sb[:],
                                            mf)

            def emit_corr(cc):
                # corr_j = sum_{taps,i} wsc[i,j]*mean_i via N=1 PE matvecs,
                # accumulated in PSUM (uses a full-bank tile in the rotation)
                s = st[cc]
                wsc = s["wsc"]
                mb = s["mb"]
                cp = pspool.tile([128, 512], f32, tag="pb", name=f"cp{cc}")
                for ti in range(NTAP):
                    start, stop = (ti == 0), (ti == NTAP - 1)
                    for R in range(2):
                        lhsT = wsc[64 * R:64 * R + 64, ti * 64: ti * 64 + 64]
                        for Cg in range(2):
                            nc.tensor.matmul(
                                cp[64 * Cg:64 * Cg + 64, R:R + 1], lhsT,
                                mb[64 * R:64 * R + 64, 0:1],
                                start=start, stop=stop,
                                tile_position=(64 * R, 64 * Cg))
                s["corrp"] = cp

            def emit_bc(cc):
                # bc[:, R] = bias[:, cc*2+R] - corr[:, R] on ACT (gpsimd has
                # no PSUM access): Identity(corr * -1 + bias)
                s = st[cc]
                bc = spool.tile([128, 2], f32, tag="bc", name=f"bc{cc}")
                for R in range(2):
                    nc.scalar.activation(
                        bc[:, R:R + 1], s["corrp"][:, R:R + 1], IDENT,
                        bias=bias_sb[:, cc * 2 + R:cc * 2 + R + 1], scale=-1.0)
                s["bc"] = bc

            def emit_window_mms(cc, w):
                # window = spans (2w, 2w+1); tap-outer so one lhsT pair serves
                # 8 matmuls.  span q covers spatial tiles {q, 16+q} (Cg).
                xt = st[cc]["xt"]
                wsc = st[cc]["wsc"]
                qs = (2 * w, 2 * w + 1)
                pb = {q: [pspool.tile([128, 512], f32, tag="pb",
                                      name=f"pb{cc}_{q}_{R}") for R in range(2)]
                      for q in qs}
                for ti, (dh, dwi) in enumerate(TAPS):
                    start, stop = (ti == 0), (ti == NTAP - 1)
                    tapi = dh * 3 + dwi
                    for R in range(2):
                        lhsT = wsc[64 * R:64 * R + 64,
                                   tapi * 64: tapi * 64 + 64]
                        for q in qs:
                            for Cg in range(2):
                                t = 16 * Cg + q
                                base = (4 * t + dh) * W
                                outp = pb[q][R][64 * Cg:64 * Cg + 64, :]
                                tp = (64 * R, 64 * Cg)
                                if dwi == 1:
                                    nc.tensor.matmul(
                                        outp, lhsT,
                                        xt[64 * R:64 * R + 64, base:base + 512],
                                        start=start, stop=stop,
                                        tile_position=tp)
                                else:
                                    o3 = outp.rearrange("p (h w) -> p h w", w=W)
                                    r3 = xt[64 * R:64 * R + 64,
                                            base:base + 512].rearrange(
                                                "p (h w) -> p h w", w=W)
                                    if dwi == 0:   # dw=-1
                                        nc.tensor.matmul(
                                            o3[:, :, 1:W], lhsT,
                                            r3[:, :, 0:W - 1],
                                            start=start, stop=stop,
                                            skip_group_check=True,
                                            tile_position=tp)
                                    else:          # dw=+1
                                        nc.tensor.matmul(
                                            o3[:, :, 0:W - 1], lhsT,
                                            r3[:, :, 1:W],
                                            start=start, stop=stop,
                                            skip_group_check=True,
                                            tile_position=tp)
                return pb

            def emit_evac(cc, q, pb_q, om, dve=False):
                # om = psum + bc (bias with folded -corr), f32 -> bf16.
                # Mostly ACT (fastest); a couple of windows' R=1 go to DVE to
                # balance engine load.
                bc = st[cc]["bc"]
                for R in range(2):
                    dst = om[:, R * 8192 + q * 512: R * 8192 + q * 512 + 512]
                    if dve and R == 1:
                        nc.vector.tensor_scalar_add(dst, pb_q[R][:, :],
                                                    bc[:, R:R + 1])
                    else:
                        nc.scalar.activation(dst, pb_q[R][:, :], IDENT,
                                             bias=bc[:, R:R + 1])

            def emit_out(cc, om, hh):
                for Cg in range(2):
                    nc.sync.dma_start(
                        out_v[cc, Cg, hh],
                        om[64 * Cg:64 * Cg + 64, :].rearrange(
                            "p (R hh e) -> p R hh e", hh=2, e=4096)[:, :, hh, :])

            # finer store view for the last chunk's drains (shrinks the tail)
            out_v4 = out_d[:].rearrange("(a R p) (Ch qq e) -> a Ch qq p R e",
                                        a=NCH, R=2, p=64, Ch=2, qq=4, e=2048)

            def emit_out4(cc, om, part):
                for Cg in range(2):
                    nc.sync.dma_start(
                        out_v4[cc, Cg, part],
                        om[64 * Cg:64 * Cg + 64, :].rearrange(
                            "p (R qq e) -> p R qq e", qq=4, e=2048)[:, :, part, :])

            # ---- prologue: chunks 0,1 loads+stats; chunk-0 chain up front.
            # ACT takes slice 0 (lands first) so the slice-3 landing gates
            # only the (faster) DVE share; wscale(0) runs on ACT so it is
            # not queued behind chunk 1's stats on DVE.
            emit_load(0)
            for k in range(3):
                emit_stats_dve(0, k)
            emit_stats_act(0)
            emit_aggr(0)
            emit_load(1)
            for k in range(3):
                emit_stats_dve(1, k)
            emit_stats_act(1)
            emit_mix(0)
            emit_chain(0)
            emit_wscale(0, on_gp=True)
            emit_halo(0)
            emit_corr(0)
            emit_bc(0)

            # ---- steady state: per chunk, 8 tap-outer windows of 2 spans.
            # chain(cc+1) is emitted early (w==1) off already-complete stats;
            # stats(cc+2) ride the DVE queue as load slices land.
            for cc in range(NCH):
                om = opool.tile([128, 4 * 4096], bf16, tag="om", name=f"om{cc}")
                for w in range(8):
                    pb = emit_window_mms(cc, w)
                    for q in (2 * w, 2 * w + 1):
                        emit_evac(cc, q, pb[q], om, dve=(w in (3, 6)))
                    if w == 0 and cc + 2 < NCH:
                        emit_load(cc + 2)
                    if w == 1 and cc + 1 < NCH:
                        emit_aggr(cc + 1)
                        emit_mix(cc + 1)
                        emit_chain(cc + 1)
                        emit_wscale(cc + 1)
                        emit_halo(cc + 1)
                    if w in (2, 3, 4) and cc + 2 < NCH:
                        emit_stats_dve(cc + 2, w - 2)
                    if w == 5 and cc + 2 < NCH:
                        emit_stats_act(cc + 2)
                    if cc < NCH - 1:
                        if w == 3:
                            emit_out(cc, om, 0)
                    elif w in (1, 3, 5):
                        emit_out4(cc, om, w // 2)
                if cc + 1 < NCH:
                    emit_corr(cc + 1)
                    emit_bc(cc + 1)
                if cc < NCH - 1:
                    emit_out(cc, om, 1)
                else:
                    emit_out4(cc, om, 3)
    nc.compile()
    return nc


def _pack_inputs(x, dw, pw, biases):
    """Host-side: fuse pw o dw, scatter into block-diag 64x64 lhsT tiles."""
    G = 128
    dwr = dw.reshape(B, G, 4, 4, 3, 3)          # [b, g, m, i, kh, kw]
    pwr = pw.reshape(B, G, 4, 4)                # [b, g, j, m]
    eff = np.einsum('bgjm,bgmikl->bgjikl', pwr, dwr)  # [b, g, j, i, kh, kw]
    # 64x64 block-diag tiles: w_host[b, 64R + 4gl + i, (cc*9+tap)*64 + 4gl + j]
    w_host = np.zeros((B, 128, NCH * NTAP * 64), dtype=np.float32)
    wv = w_host.reshape(B, 2, 16, 4, NCH, NTAP, 16, 4)  # [b,R,gl_k,i,cc,tap,gl_m,j]
    er = eff.reshape(B, NCH, 2, 16, 4, 4, NTAP)         # [b, cc, R, gl, j, i, tap]
    for gl in range(16):
        e = er[:, :, :, gl]                     # [b, cc, R, j, i, tap]
        wv[:, :, gl, :, :, :, gl, :] = e.transpose(0, 2, 4, 1, 5, 3)
    bias_host = np.zeros((B, 128, 8), dtype=np.float32)
    bfull = biases.reshape(B, C)
    p = np.arange(128)
    for cc in range(NCH):
        for R in range(2):
            bias_host[:, :, cc * 2 + R] = bfull[:, cc * 128 + 64 * R + (p % 64)]
    return w_host.astype(ml_dtypes.bfloat16), bias_host


def kernel(x, dw_kernels, pw_kernels, biases):
    from concourse.bass_utils import run_bass_kernel_spmd

    x = np.ascontiguousarray(np.asarray(x, dtype=np.float32))
    dw = np.asarray(dw_kernels, dtype=np.float32)
    pw = np.asarray(pw_kernels, dtype=np.float32)
    bs = np.asarray(biases, dtype=np.float32)

    if "nc" not in _CACHE:
        _CACHE["nc"] = _build_program()
    nc = _CACHE["nc"]

    w_host, bias_host = _pack_inputs(x, dw, pw, bs)
    xb = x.reshape(B, C, HW).astype(ml_dtypes.bfloat16)
    in_maps = [{"x": xb[i],
                "w": w_host[i],
                "bias": bias_host[i]} for i in range(B)]
    res = run_bass_kernel_spmd(nc, in_maps, core_ids=list(range(B)),
                               trace=bool(int(os.environ.get("KTRACE", "0"))))
    _CACHE["last_result"] = res
    out = np.stack([res.results[i]["out"].astype(np.float32).reshape(C, H, W)
                    for i in range(B)])
    return out


# revision 29
# speedup vs baseline: 1.1865x; 1.1865x over previous
"""AdaConv2D (instance-norm -> grouped 3x3 conv -> grouped 1x1 conv -> bias) on 8 TRN2 cores.

v2 strategy (pure data parallel, 1 sample/core, no collectives).  Key change
vs v1 (195.5us): the instance-norm is FOLDED INTO THE WEIGHTS instead of
materialized as a normalize pass over x:

  out = conv(xn) + b,  xn = (x - mean)*rstd
      = conv_{w*rstd}(x_raw) + (b - sum_taps w*rstd*mean)

  - Device computes mean/rstd per channel (DVE bn_stats one pass + tiny ACT
    Ln/Exp chain), scales the chunk's weights by rstd (one ACT op, 576
    elems/partition), and runs the conv directly on RAW bf16 x.
  - H-edges: the 1-row top/bottom halo is filled with the per-channel MEAN
    (not zero): out-of-image taps then contribute w*mean which exactly
    cancels against the folded bias correction.
  - Bias correction corr_j = sum_{i,9 taps} w_s[i,j]*mean_i is computed on
    the PE itself: 36 N=1 matvec accumulations (4 quadrants x 9 taps) into a
    PSUM tile, ~1us/chunk.  bc = bias - corr (gpsimd).
  - W-edges keep the shrunk-AP taps; their (sub-1e-3) bias-corr discrepancy
    is left uncorrected - measured total rel err 1.2e-3 vs budget 2e-2.

  This removes the entire normalize pass (was ~17us/chunk of gpsimd/DVE) and
  the load->stats->chain->normalize critical path that held the first matmul
  until 41us and caused 16us of mid-kernel PE stalls.

  Engine layout per 128-channel chunk (4 chunks/sample):
  - Sync (HWDGE): all DMA. in: 4x1MB slices, two chunks ahead; out: 1MB
    chunks per spatial half (quarters on the last chunk).
  - DVE: bn_stats x32 (one pass over x as slices land) + bn_aggr.
  - ACT: rstd chain (Ln/Exp), mean->bf16, weight scaling, and ALL PSUM
    eviction (activation Identity with per-partition bias AP, psum f32 ->
    bf16 staging; ACT is the fastest eviction engine at ~0.71ns/elem).
  - GpSimd: halo fills (2 broadcast ops) + bc = bias - corr.  Nothing bulk.
  - TensorE: conv as 4 concurrent 64x64 tile_position matmuls (2 channel
    sub-chunk PAIRS x 2 spatial halves), 9 taps accumulated in PSUM.
    TAP-OUTER over windows of 2 spans: per tap one pair of lhsT loads serves
    8 matmuls (2 spans x 4 quadrants), halving exposure to the 64-col
    LDWEIGHTS serialization that bounds v1 (~212ns/tap of weight load vs
    ~107ns of bf16 moving-operand streaming).
  - PSUM: single 8-buf pool; each window holds 4 banks, corr rides the same
    rotation as a full-bank tile once per chunk.
"""
import os
import sys
import numpy as np
import ml_dtypes

if "/opt/trn_rl_repo" not in sys.path:
    sys.path.insert(0, "/opt/trn_rl_repo")

B, C, H, W = 8, 512, 128, 128
HW = H * W            # 16384
NCH = 4               # 128-channel chunks per sample
NTAP = 9
ROWS_PAD = H + 2      # 130 rows of 128 in padded SBUF layout
PADF = ROWS_PAD * W   # 16640 elems per partition
# taps ordered so the first three are dw=0 (full-width writes -> correct PSUM init)
TAPS = [(0, 1), (1, 1), (2, 1), (0, 0), (1, 0), (2, 0), (0, 2), (1, 2), (2, 2)]

_CACHE = {}


def _build_program():
    import concourse.bass as bass
    import concourse.tile as tile
    from concourse import bacc, mybir

    f32 = mybir.dt.float32
    bf16 = mybir.dt.bfloat16
    COPY = mybir.ActivationFunctionType.Copy
    IDENT = mybir.ActivationFunctionType.Identity
    SQRT = mybir.ActivationFunctionType.Sqrt
    EPS = 1e-7
    nc = bacc.Bacc("TRN2", target_bir_lowering=False, debug=False,
                   enable_asserts=False, num_devices=8)

    x_d = nc.dram_tensor("x", [C, HW], bf16, kind="ExternalInput")
    w_d = nc.dram_tensor("w", [128, NCH * NTAP * 64], bf16, kind="ExternalInput")
    b_d = nc.dram_tensor("bias", [128, 8], f32, kind="ExternalInput")
    out_d = nc.dram_tensor("out", [C, HW], bf16, kind="ExternalOutput")

    # store view: [cc, Ch(spatial half), hh(drain half), p, R, e(4096)]
    out_v = out_d[:].rearrange("(a R p) (Ch hh e) -> a Ch hh p R e", a=NCH, R=2,
                               p=64, Ch=2, hh=2, e=4096)

    with tile.TileContext(nc) as tc:
        with (
            tc.tile_pool(name="xpool", bufs=3) as xpool,
            tc.tile_pool(name="wpool", bufs=1) as wpool,
            tc.tile_pool(name="spool", bufs=3) as spool,
            tc.tile_pool(name="opool", bufs=2) as opool,
            tc.tile_pool(name="psum", bufs=8, space=bass.MemorySpace.PSUM) as pspool,
        ):
            w_sb = wpool.tile([128, NCH * NTAP * 64], bf16)
            nc.sync.dma_start(w_sb[:], w_d[:])
            bias_sb = wpool.tile([128, 8], f32)
            nc.sync.dma_start(bias_sb[:], b_d[:])
            ones_sb = wpool.tile([128, W], bf16)
            nc.gpsimd.memset(ones_sb[:], 1.0)
            trash0 = wpool.tile([128, 4096], bf16)
            trash1 = wpool.tile([128, 4096], bf16)

            st = {}  # per-chunk small tiles

            def emit_load(cc):
                xt = xpool.tile([128, PADF], bf16, tag="xt", name=f"xt{cc}")
                st[cc] = {"xt": xt}
                for k in range(2):
                    nc.sync.dma_start(xt[:, W + k * 8192: W + (k + 1) * 8192],
                                      x_d[cc * 128:(cc + 1) * 128,
                                          k * 8192:(k + 1) * 8192])

            def emit_stats_dve(cc, k, pos=None):
                # DVE bn_stats over slice k (8 blocks of 512), single pass;
                # pos = 8-block position within the 24-block stats6 tile
                s = st[cc]
                if pos is None:
                    pos = k
                if pos == 0:
                    s["stats6"] = spool.tile([128, 24 * 6], f32, tag="stats",
                                             name=f"st{cc}")
                xt = s["xt"]
                for jj in range(8):
                    j = 8 * k + jj
                    o = (8 * pos + jj) * 6
                    nc.vector.bn_stats(s["stats6"][:, o:o + 6],
                                       xt[:, W + j * 512: W + (j + 1) * 512])

            def emit_stats_act(cc, k=3):
                # ACT: sum + sumsq of slice k via Copy/Square accumulators
                s = st[cc]
                acc = spool.tile([128, 2], f32, tag="acc", name=f"ac{cc}")
                sl = s["xt"][:, W + k * 4096: W + (k + 1) * 4096]
                nc.scalar.activation(trash0[:], sl, COPY,
                                     accum_out=acc[:, 0:1])
                nc.scalar.activation(trash1[:], sl,
                                     mybir.ActivationFunctionType.Square,
                                     accum_out=acc[:, 1:2])
                s["acc"] = acc

            def emit_aggr(cc):
                s = st[cc]
                mv = spool.tile([128, 2], f32, tag="mv", name=f"mv{cc}")
                nc.vector.bn_aggr(mv[:], s["stats6"][:].rearrange(
                    "p (h s) -> p h s", s=6))
                s["mv24"] = mv

            def emit_mix(cc):
                # gpsimd: merge DVE 24-block stats (3/4 of chunk) with the
                # ACT accumulators (1/4): mean, var, all [128,1] f32 ops
                s = st[cc]
                g = nc.gpsimd
                ADD = mybir.AluOpType.add
                MUL = mybir.AluOpType.mult
                mv = s["mv24"]
                acc = s["acc"]
                t0 = spool.tile([128, 2], f32, tag="t0", name=f"t0{cc}")
                g.tensor_scalar_mul(t0[:], acc[:], 1.0 / HW)
                t1 = spool.tile([128, 2], f32, tag="t1", name=f"t1{cc}")
                g.tensor_scalar_mul(t1[:], mv[:], 0.75)  # [.75 m24, .75 v24]
                mean = spool.tile([128, 1], f32, tag="mean", name=f"me{cc}")
                g.tensor_add(mean[:], t1[:, 0:1], t0[:, 0:1])
                m2a = spool.tile([128, 1], f32, tag="m2a", name=f"ma{cc}")
                g.tensor_mul(m2a[:], t1[:, 0:1], mv[:, 0:1])  # .75 m24^2
                e2a = spool.tile([128, 1], f32, tag="e2a", name=f"ea{cc}")
                g.tensor_add(e2a[:], t1[:, 1:2], m2a[:])
                ex2 = spool.tile([128, 1], f32, tag="ex2", name=f"ex{cc}")
                g.tensor_add(ex2[:], e2a[:], t0[:, 1:2])
                m2 = spool.tile([128, 1], f32, tag="m2", name=f"m2{cc}")
                g.tensor_mul(m2[:], mean[:], mean[:])
                var = spool.tile([128, 1], f32, tag="var", name=f"va{cc}")
                g.tensor_sub(var[:], ex2[:], m2[:])
                s["mean"] = mean
                s["var"] = var

            def emit_chain(cc):
                # rstd = 1/(sqrt(var*N/(N-1)) + eps).  Sqrt/Copy/Identity/
                # Square share one ACT table set -> no table reloads (Ln/Exp
                # forced a 1.3us table swap per use).
                s = st[cc]
                lg = spool.tile([128, 1], f32, tag="lg", name=f"lg{cc}")
                nc.scalar.activation(lg[:], s["var"][:],
                                     mybir.ActivationFunctionType.Ln,
                                     scale=float(HW) / float(HW - 1))
                rstd = spool.tile([128, 1], f32, tag="rstd", name=f"rs{cc}")
                nc.scalar.activation(rstd[:], lg[:],
                                     mybir.ActivationFunctionType.Exp,
                                     scale=-0.5)
                mb = spool.tile([128, 1], bf16, tag="mb", name=f"mb{cc}")
                nc.scalar.activation(mb[:], s["mean"][:], COPY)
                s["rstd"] = rstd
                s["mb"] = mb

            def emit_wscale(cc, on_gp=False):
                # scale this chunk's weights by rstd (per input channel row).
                # chunk 0 runs on the idle gpsimd queue so it is not stuck
                # behind chunk 1's bn_stats on DVE in the prologue.
                s = st[cc]
                wsc = spool.tile([128, NTAP * 64], bf16, tag="wsc",
                                 name=f"ws{cc}")
                src = w_sb[:, cc * NTAP * 64:(cc + 1) * NTAP * 64]
                eng = nc.gpsimd if on_gp else nc.vector
                eng.tensor_scalar_mul(wsc[:], src, s["rstd"][:, 0:1])
                s["wsc"] = wsc

            def emit_halo(cc):
                # top/bottom halo row := per-channel mean (bf16) so that
                # out-of-image dh taps cancel against the folded bias corr.
                # scalar operand must be f32; the bf16 output rounds to the
                # same value as the mb tile used by the corr matvec.
                s = st[cc]
                xt = s["xt"]
                mf = s["mean"][:, 0:1]
                nc.gpsimd.tensor_scalar_mul(xt[:, 0:W], ones_sb[:], mf)
                nc.gpsimd.tensor_scalar_mul(xt[:, PADF - W:PADF], ones_sb[:],
                                            mf)

            def emit_corr(cc):
                # corr_j = sum_{taps,i} wsc[i,j]*mean_i via N=1 PE matvecs,
                # accumulated in PSUM (uses a full-bank tile in the rotation)
                s = st[cc]
                wsc = s["wsc"]
                mb = s["mb"]
                cp = pspool.tile([128, 512], f32, tag="pb", name=f"cp{cc}")
                for ti in range(NTAP):
                    start, stop = (ti == 0), (ti == NTAP - 1)
                    for R in range(2):
                        lhsT = wsc[64 * R:64 * R + 64, ti * 64: ti * 64 + 64]
                        for Cg in range(2):
                            nc.tensor.matmul(
                                cp[64 * Cg:64 * Cg + 64, R:R + 1], lhsT,
                                mb[64 * R:64 * R + 64, 0:1],
                                start=start, stop=stop,
                                tile_position=(64 * R, 64 * Cg))
                s["corrp"] = cp

            def emit_bc(cc):
                # bc[:, R] = bias[:, cc*2+R] - corr[:, R] on ACT (gpsimd has
                # no PSUM access): Identity(corr * -1 + bias)
                s = st[cc]
                bc = spool.tile([128, 2], f32, tag="bc", name=f"bc{cc}")
                for R in range(2):
                    nc.scalar.activation(
                        bc[:, R:R + 1], s["corrp"][:, R:R + 1], IDENT,
                        bias=bias_sb[:, cc * 2 + R:cc * 2 + R + 1], scale=-1.0)
                s["bc"] = bc

            def emit_window_mms(cc, w):
                # window = spans (2w, 2w+1); tap-outer so one lhsT pair serves
                # 8 matmuls.  span q covers spatial tiles {q, 16+q} (Cg).
                xt = st[cc]["xt"]
                wsc = st[cc]["wsc"]
                qs = (2 * w, 2 * w + 1)
                pb = {q: [pspool.tile([128, 512], f32, tag="pb",
                                      name=f"pb{cc}_{q}_{R}") for R in range(2)]
                      for q in qs}
                for ti, (dh, dwi) in enumerate(TAPS):
                    start, stop = (ti == 0), (ti == NTAP - 1)
                    tapi = dh * 3 + dwi
                    for R in range(2):
                        lhsT = wsc[64 * R:64 * R + 64,
                                   tapi * 64: tapi * 64 + 64]
                        for q in qs:
                            for Cg in range(2):
                                t = 16 * Cg + q
                                base = (4 * t + dh) * W
                                outp = pb[q][R][64 * Cg:64 * Cg + 64, :]
                                tp = (64 * R, 64 * Cg)
                                if dwi == 1:
                                    nc.tensor.matmul(
                                        outp, lhsT,
                                        xt[64 * R:64 * R + 64, base:base + 512],
                                        start=start, stop=stop,
                                        tile_position=tp)
                                else:
                                    o3 = outp.rearrange("p (h w) -> p h w", w=W)
                                    r3 = xt[64 * R:64 * R + 64,
                                            base:base + 512].rearrange(
                                                "p (h w) -> p h w", w=W)
                                    if dwi == 0:   # dw=-1
                                        nc.tensor.matmul(
                                            o3[:, :, 1:W], lhsT,
                                            r3[:, :, 0:W - 1],
                                            start=start, stop=stop,
                                            skip_group_check=True,
                                            tile_position=tp)
                                    else:          # dw=+1
                                        nc.tensor.matmul(
                                            o3[:, :, 0:W - 1], lhsT,
                                            r3[:, :, 1:W],
                                            start=start, stop=stop,
                                            skip_group_check=True,
                                            tile_position=tp)
                return pb

            def emit_evac(cc, q, pb_q, om, dve=False):
                # om = psum + bc (bias with folded -corr), f32 -> bf16.
                # Mostly ACT (fastest); a couple of windows' R=1 go to DVE to
                # balance engine load.
                bc = st[cc]["bc"]
                for R in range(2):
                    dst = om[:, R * 8192 + q * 512: R * 8192 + q * 512 + 512]
                    if dve and R == 1:
                        nc.vector.tensor_scalar_add(dst, pb_q[R][:, :],
                                                    bc[:, R:R + 1])
                    else:
                        nc.scalar.activation(dst, pb_q[R][:, :], IDENT,
                                             bias=bc[:, R:R + 1])

            def emit_out(cc, om, hh):
                for Cg in range(2):
                    nc.sync.dma_start(
                        out_v[cc, Cg, hh],
                        om[64 * Cg:64 * Cg + 64, :].rearrange(
                            "p (R hh e) -> p R hh e", hh=2, e=4096)[:, :, hh, :])

            # finer store view for the last chunk's drains (shrinks the tail)
            out_v4 = out_d[:].rearrange("(a R p) (Ch qq e) -> a Ch qq p R e",
                                        a=NCH, R=2, p=64, Ch=2, qq=4, e=2048)

            def emit_out4(cc, om, part):
                for Cg in range(2):
                    nc.sync.dma_start(
                        out_v4[cc, Cg, part],
                        om[64 * Cg:64 * Cg + 64, :].rearrange(
                            "p (R qq e) -> p R qq e", qq=4, e=2048)[:, :, part, :])

            # ---- prologue: chunks 0,1 loads+stats; chunk-0 chain up front.
            # ACT takes slice 0 (lands first) so the slice-3 landing gates
            # only the (faster) DVE share; wscale(0) runs on ACT so it is
            # not queued behind chunk 1's stats on DVE.
            emit_load(0)
            for k in range(3):
                emit_stats_dve(0, k)
            emit_stats_act(0)
            emit_aggr(0)
            emit_load(1)
            for k in range(3):
                emit_stats_dve(1, k)
            emit_stats_act(1)
            emit_mix(0)
            emit_chain(0)
            emit_wscale(0)
            emit_halo(0)
            emit_corr(0)
            emit_bc(0)

            # ---- steady state: per chunk, 8 tap-outer windows of 2 spans.
            # chain(cc+1) is emitted early (w==1) off already-complete stats;
            # stats(cc+2) ride the DVE queue as load slices land.
            for cc in range(NCH):
                om = opool.tile([128, 4 * 4096], bf16, tag="om", name=f"om{cc}")
                for w in range(8):
                    pb = emit_window_mms(cc, w)
                    for q in (2 * w, 2 * w + 1):
                        emit_evac(cc, q, pb[q], om, dve=(w in (3, 6)))
                    if w == 0 and cc + 2 < NCH:
                        emit_load(cc + 2)
                    if w == 1 and cc + 1 < NCH:
                        emit_aggr(cc + 1)
                        emit_mix(cc + 1)
                        emit_chain(cc + 1)
                        emit_wscale(cc + 1)
                        emit_halo(cc + 1)
                    if w in (2, 3, 4) and cc + 2 < NCH:
                        emit_stats_dve(cc + 2, w - 2)
                    if w == 5 and cc + 2 < NCH:
                        emit_stats_act(cc + 2)
                    if cc < NCH - 1:
                        if w == 3:
                            emit_out(cc, om, 0)
                    elif w in (1, 3, 5):
                        emit_out4(cc, om, w // 2)
                if cc + 1 < NCH:
                    emit_corr(cc + 1)
                    emit_bc(cc + 1)
                if cc < NCH - 1:
                    emit_out(cc, om, 1)
                else:
                    emit_out4(cc, om, 3)
    nc.compile()
    return nc


def _pack_inputs(x, dw, pw, biases):
    """Host-side: fuse pw o dw, scatter into block-diag 64x64 lhsT tiles."""
    G = 128
    dwr = dw.reshape(B, G, 4, 4, 3, 3)          # [b, g, m, i, kh, kw]
    pwr = pw.reshape(B, G, 4, 4)                # [b, g, j, m]
    eff = np.einsum('bgjm,bgmikl->bgjikl', pwr, dwr)  # [b, g, j, i, kh, kw]
    # 64x64 block-diag tiles: w_host[b, 64R + 4gl + i, (cc*9+tap)*64 + 4gl + j]
    w_host = np.zeros((B, 128, NCH * NTAP * 64), dtype=np.float32)
    wv = w_host.reshape(B, 2, 16, 4, NCH, NTAP, 16, 4)  # [b,R,gl_k,i,cc,tap,gl_m,j]
    er = eff.reshape(B, NCH, 2, 16, 4, 4, NTAP)         # [b, cc, R, gl, j, i, tap]
    for gl in range(16):
        e = er[:, :, :, gl]                     # [b, cc, R, j, i, tap]
        wv[:, :, gl, :, :, :, gl, :] = e.transpose(0, 2, 4, 1, 5, 3)
    bias_host = np.zeros((B, 128, 8), dtype=np.float32)
    bfull = biases.reshape(B, C)
    p = np.arange(128)
    for cc in range(NCH):
        for R in range(2):
            bias_host[:, :, cc * 2 + R] = bfull[:, cc * 128 + 64 * R + (p % 64)]
    return w_host.astype(ml_dtypes.bfloat16), bias_host


def kernel(x, dw_kernels, pw_kernels, biases):
    from concourse.bass_utils import run_bass_kernel_spmd

    x = np.ascontiguousarray(np.asarray(x, dtype=np.float32))
    dw = np.asarray(dw_kernels, dtype=np.float32)
    pw = np.asarray(pw_kernels, dtype=np.float32)
    bs = np.asarray(biases, dtype=np.float32)

    if "nc" not in _CACHE:
        _CACHE["nc"] = _build_program()
    nc = _CACHE["nc"]

    w_host, bias_host = _pack_inputs(x, dw, pw, bs)
    xb = x.reshape(B, C, HW).astype(ml_dtypes.bfloat16)
    in_maps = [{"x": xb[i],
                "w": w_host[i],
                "bias": bias_host[i]} for i in range(B)]
    res = run_bass_kernel_spmd(nc, in_maps, core_ids=list(range(B)),
                               trace=bool(int(os.environ.get("KTRACE", "0"))))
    _CACHE["last_result"] = res
    out = np.stack([res.results[i]["out"].astype(np.float32).reshape(C, H, W)
                    for i in range(B)])
    return out


# revision 31
# speedup vs baseline: 1.1906x; 1.0034x over previous
"""AdaConv2D (instance-norm -> grouped 3x3 conv -> grouped 1x1 conv -> bias) on 8 TRN2 cores.

v2 strategy (pure data parallel, 1 sample/core, no collectives).  Key change
vs v1 (195.5us): the instance-norm is FOLDED INTO THE WEIGHTS instead of
materialized as a normalize pass over x:

  out = conv(xn) + b,  xn = (x - mean)*rstd
      = conv_{w*rstd}(x_raw) + (b - sum_taps w*rstd*mean)

  - Device computes mean/rstd per channel (DVE bn_stats one pass + tiny ACT
    Ln/Exp chain), scales the chunk's weights by rstd (one ACT op, 576
    elems/partition), and runs the conv directly on RAW bf16 x.
  - H-edges: the 1-row top/bottom halo is filled with the per-channel MEAN
    (not zero): out-of-image taps then contribute w*mean which exactly
    cancels against the folded bias correction.
  - Bias correction corr_j = sum_{i,9 taps} w_s[i,j]*mean_i is computed on
    the PE itself: 36 N=1 matvec accumulations (4 quadrants x 9 taps) into a
    PSUM tile, ~1us/chunk.  bc = bias - corr (gpsimd).
  - W-edges keep the shrunk-AP taps; their (sub-1e-3) bias-corr discrepancy
    is left uncorrected - measured total rel err 1.2e-3 vs budget 2e-2.

  This removes the entire normalize pass (was ~17us/chunk of gpsimd/DVE) and
  the load->stats->chain->normalize critical path that held the first matmul
  until 41us and caused 16us of mid-kernel PE stalls.

  Engine layout per 128-channel chunk (4 chunks/sample):
  - Sync (HWDGE): all DMA. in: 4x1MB slices, two chunks ahead; out: 1MB
    chunks per spatial half (quarters on the last chunk).
  - DVE: bn_stats x32 (one pass over x as slices land) + bn_aggr.
  - ACT: rstd chain (Ln/Exp), mean->bf16, weight scaling, and ALL PSUM
    eviction (activation Identity with per-partition bias AP, psum f32 ->
    bf16 staging; ACT is the fastest eviction engine at ~0.71ns/elem).
  - GpSimd: halo fills (2 broadcast ops) + bc = bias - corr.  Nothing bulk.
  - TensorE: conv as 4 concurrent 64x64 tile_position matmuls (2 channel
    sub-chunk PAIRS x 2 spatial halves), 9 taps accumulated in PSUM.
    TAP-OUTER over windows of 2 spans: per tap one pair of lhsT loads serves
    8 matmuls (2 spans x 4 quadrants), halving exposure to the 64-col
    LDWEIGHTS serialization that bounds v1 (~212ns/tap of weight load vs
    ~107ns of bf16 moving-operand streaming).
  - PSUM: single 8-buf pool; each window holds 4 banks, corr rides the same
    rotation as a full-bank tile once per chunk.
"""
import os
import sys
import numpy as np
import ml_dtypes

if "/opt/trn_rl_repo" not in sys.path:
    sys.path.insert(0, "/opt/trn_rl_repo")

B, C, H, W = 8, 512, 128, 128
HW = H * W            # 16384
NCH = 4               # 128-channel chunks per sample
NTAP = 9
ROWS_PAD = H + 2      # 130 rows of 128 in padded SBUF layout
PADF = ROWS_PAD * W   # 16640 elems per partition
# taps ordered so the first three are dw=0 (full-width writes -> correct PSUM init)
TAPS = [(0, 1), (1, 1), (2, 1), (0, 0), (1, 0), (2, 0), (0, 2), (1, 2), (2, 2)]

_CACHE = {}


def _build_program():
    import concourse.bass as bass
    import concourse.tile as tile
    from concourse import bacc, mybir

    f32 = mybir.dt.float32
    bf16 = mybir.dt.bfloat16
    COPY = mybir.ActivationFunctionType.Copy
    IDENT = mybir.ActivationFunctionType.Identity
    SQRT = mybir.ActivationFunctionType.Sqrt
    EPS = 1e-7
    nc = bacc.Bacc("TRN2", target_bir_lowering=False, debug=False,
                   enable_asserts=False, num_devices=8)

    x_d = nc.dram_tensor("x", [C, HW], bf16, kind="ExternalInput")
    w_d = nc.dram_tensor("w", [128, NCH * NTAP * 64], bf16, kind="ExternalInput")
    b_d = nc.dram_tensor("bias", [128, 8], f32, kind="ExternalInput")
    out_d = nc.dram_tensor("out", [C, HW], bf16, kind="ExternalOutput")

    # store view: [cc, Ch(spatial half), hh(drain half), p, R, e(4096)]
    out_v = out_d[:].rearrange("(a R p) (Ch hh e) -> a Ch hh p R e", a=NCH, R=2,
                               p=64, Ch=2, hh=2, e=4096)

    with tile.TileContext(nc) as tc:
        with (
            tc.tile_pool(name="xpool", bufs=3) as xpool,
            tc.tile_pool(name="wpool", bufs=1) as wpool,
            tc.tile_pool(name="spool", bufs=3) as spool,
            tc.tile_pool(name="opool", bufs=2) as opool,
            tc.tile_pool(name="psum", bufs=8, space=bass.MemorySpace.PSUM) as pspool,
        ):
            # Stage weights/bias through a one-shot copy: the DMA sem then
            # has a single early consumer, so its reuse-reset does not wait
            # for chunk-3-era wscale/bc instructions (measured 78us sync
            # stall blocking all later DMA issues).
            w_dma = wpool.tile([128, NCH * NTAP * 64], bf16)
            nc.sync.dma_start(w_dma[:], w_d[:])
            w_sb = wpool.tile([128, NCH * NTAP * 64], bf16)
            nc.vector.tensor_scalar_mul(w_sb[:], w_dma[:], 1.0)
            bias_dma = wpool.tile([128, 8], f32)
            nc.sync.dma_start(bias_dma[:], b_d[:])
            bias_sb = wpool.tile([128, 8], f32)
            nc.vector.tensor_scalar_mul(bias_sb[:], bias_dma[:], 1.0)
            ones_sb = wpool.tile([128, W], bf16)
            nc.gpsimd.memset(ones_sb[:], 1.0)
            trash0 = wpool.tile([128, 4096], bf16)
            trash1 = wpool.tile([128, 4096], bf16)

            st = {}  # per-chunk small tiles

            def emit_load(cc):
                xt = xpool.tile([128, PADF], bf16, tag="xt", name=f"xt{cc}")
                st[cc] = {"xt": xt}
                for k in range(2):
                    nc.sync.dma_start(xt[:, W + k * 8192: W + (k + 1) * 8192],
                                      x_d[cc * 128:(cc + 1) * 128,
                                          k * 8192:(k + 1) * 8192])

            def emit_stats_dve(cc, k, pos=None):
                # DVE bn_stats over slice k (8 blocks of 512), single pass;
                # pos = 8-block position within the 24-block stats6 tile
                s = st[cc]
                if pos is None:
                    pos = k
                if pos == 0:
                    s["stats6"] = spool.tile([128, 24 * 6], f32, tag="stats",
                                             name=f"st{cc}")
                xt = s["xt"]
                for jj in range(8):
                    j = 8 * k + jj
                    o = (8 * pos + jj) * 6
                    nc.vector.bn_stats(s["stats6"][:, o:o + 6],
                                       xt[:, W + j * 512: W + (j + 1) * 512])

            def emit_stats_act(cc, k=3):
                # ACT: sum + sumsq of slice k via Copy/Square accumulators
                s = st[cc]
                acc = spool.tile([128, 2], f32, tag="acc", name=f"ac{cc}")
                sl = s["xt"][:, W + k * 4096: W + (k + 1) * 4096]
                nc.scalar.activation(trash0[:], sl, COPY,
                                     accum_out=acc[:, 0:1])
                nc.scalar.activation(trash1[:], sl,
                                     mybir.ActivationFunctionType.Square,
                                     accum_out=acc[:, 1:2])
                s["acc"] = acc

            def emit_aggr(cc):
                s = st[cc]
                mv = spool.tile([128, 2], f32, tag="mv", name=f"mv{cc}")
                nc.vector.bn_aggr(mv[:], s["stats6"][:].rearrange(
                    "p (h s) -> p h s", s=6))
                s["mv24"] = mv

            def emit_mix(cc):
                # gpsimd: merge DVE 24-block stats (3/4 of chunk) with the
                # ACT accumulators (1/4): mean, var, all [128,1] f32 ops
                s = st[cc]
                g = nc.gpsimd
                ADD = mybir.AluOpType.add
                MUL = mybir.AluOpType.mult
                mv = s["mv24"]
                acc = s["acc"]
                t0 = spool.tile([128, 2], f32, tag="t0", name=f"t0{cc}")
                g.tensor_scalar_mul(t0[:], acc[:], 1.0 / HW)
                t1 = spool.tile([128, 2], f32, tag="t1", name=f"t1{cc}")
                g.tensor_scalar_mul(t1[:], mv[:], 0.75)  # [.75 m24, .75 v24]
                mean = spool.tile([128, 1], f32, tag="mean", name=f"me{cc}")
                g.tensor_add(mean[:], t1[:, 0:1], t0[:, 0:1])
                m2a = spool.tile([128, 1], f32, tag="m2a", name=f"ma{cc}")
                g.tensor_mul(m2a[:], t1[:, 0:1], mv[:, 0:1])  # .75 m24^2
                e2a = spool.tile([128, 1], f32, tag="e2a", name=f"ea{cc}")
                g.tensor_add(e2a[:], t1[:, 1:2], m2a[:])
                ex2 = spool.tile([128, 1], f32, tag="ex2", name=f"ex{cc}")
                g.tensor_add(ex2[:], e2a[:], t0[:, 1:2])
                m2 = spool.tile([128, 1], f32, tag="m2", name=f"m2{cc}")
                g.tensor_mul(m2[:], mean[:], mean[:])
                var = spool.tile([128, 1], f32, tag="var", name=f"va{cc}")
                g.tensor_sub(var[:], ex2[:], m2[:])
                s["mean"] = mean
                s["var"] = var

            def emit_chain(cc):
                # rstd = 1/(sqrt(var*N/(N-1)) + eps).  Sqrt/Copy/Identity/
                # Square share one ACT table set -> no table reloads (Ln/Exp
                # forced a 1.3us table swap per use).
                s = st[cc]
                lg = spool.tile([128, 1], f32, tag="lg", name=f"lg{cc}")
                nc.scalar.activation(lg[:], s["var"][:],
                                     mybir.ActivationFunctionType.Ln,
                                     scale=float(HW) / float(HW - 1))
                rstd = spool.tile([128, 1], f32, tag="rstd", name=f"rs{cc}")
                nc.scalar.activation(rstd[:], lg[:],
                                     mybir.ActivationFunctionType.Exp,
                                     scale=-0.5)
                mb = spool.tile([128, 1], bf16, tag="mb", name=f"mb{cc}")
                nc.scalar.activation(mb[:], s["mean"][:], COPY)
                s["rstd"] = rstd
                s["mb"] = mb

            def emit_wscale(cc, on_gp=False):
                # scale this chunk's weights by rstd (per input channel row).
                # chunk 0 runs on the idle gpsimd queue so it is not stuck
                # behind chunk 1's bn_stats on DVE in the prologue.
                s = st[cc]
                wsc = spool.tile([128, NTAP * 64], bf16, tag="wsc",
                                 name=f"ws{cc}")
                src = w_sb[:, cc * NTAP * 64:(cc + 1) * NTAP * 64]
                eng = nc.gpsimd if on_gp else nc.vector
                eng.tensor_scalar_mul(wsc[:], src, s["rstd"][:, 0:1])
                s["wsc"] = wsc

            def emit_halo(cc):
                # top/bottom halo row := per-channel mean (bf16) so that
                # out-of-image dh taps cancel against the folded bias corr.
                # scalar operand must be f32; the bf16 output rounds to the
                # same value as the mb tile used by the corr matvec.
                s = st[cc]
                xt = s["xt"]
                mf = s["mean"][:, 0:1]
                nc.gpsimd.tensor_scalar_mul(xt[:, 0:W], ones_sb[:], mf)
                nc.gpsimd.tensor_scalar_mul(xt[:, PADF - W:PADF], ones_sb[:],
                                            mf)

            def emit_corr(cc):
                # corr_j = sum_{taps,i} wsc[i,j]*mean_i via N=1 PE matvecs,
                # accumulated in PSUM (uses a full-bank tile in the rotation)
                s = st[cc]
                wsc = s["wsc"]
                mb = s["mb"]
                cp = pspool.tile([128, 512], f32, tag="pb", name=f"cp{cc}")
                for ti in range(NTAP):
                    start, stop = (ti == 0), (ti == NTAP - 1)
                    for R in range(2):
                        lhsT = wsc[64 * R:64 * R + 64, ti * 64: ti * 64 + 64]
                        for Cg in range(2):
                            nc.tensor.matmul(
                                cp[64 * Cg:64 * Cg + 64, R:R + 1], lhsT,
                                mb[64 * R:64 * R + 64, 0:1],
                                start=start, stop=stop,
                                tile_position=(64 * R, 64 * Cg))
                s["corrp"] = cp

            def emit_bc(cc):
                # bc[:, R] = bias[:, cc*2+R] - corr[:, R] on ACT (gpsimd has
                # no PSUM access): Identity(corr * -1 + bias)
                s = st[cc]
                bc = spool.tile([128, 2], f32, tag="bc", name=f"bc{cc}")
                for R in range(2):
                    nc.scalar.activation(
                        bc[:, R:R + 1], s["corrp"][:, R:R + 1], IDENT,
                        bias=bias_sb[:, cc * 2 + R:cc * 2 + R + 1], scale=-1.0)
                s["bc"] = bc

            def emit_window_mms(cc, w):
                # window = spans (2w, 2w+1); tap-outer so one lhsT pair serves
                # 8 matmuls.  span q covers spatial tiles {q, 16+q} (Cg).
                xt = st[cc]["xt"]
                wsc = st[cc]["wsc"]
                qs = (2 * w, 2 * w + 1)
                pb = {q: [pspool.tile([128, 512], f32, tag="pb",
                                      name=f"pb{cc}_{q}_{R}") for R in range(2)]
                      for q in qs}
                for ti, (dh, dwi) in enumerate(TAPS):
                    start, stop = (ti == 0), (ti == NTAP - 1)
                    tapi = dh * 3 + dwi
                    for R in range(2):
                        lhsT = wsc[64 * R:64 * R + 64,
                                   tapi * 64: tapi * 64 + 64]
                        for q in qs:
                            for Cg in range(2):
                                t = 16 * Cg + q
                                base = (4 * t + dh) * W
                                outp = pb[q][R][64 * Cg:64 * Cg + 64, :]
                                tp = (64 * R, 64 * Cg)
                                if dwi == 1:
                                    nc.tensor.matmul(
                                        outp, lhsT,
                                        xt[64 * R:64 * R + 64, base:base + 512],
                                        start=start, stop=stop,
                                        tile_position=tp)
                                else:
                                    o3 = outp.rearrange("p (h w) -> p h w", w=W)
                                    r3 = xt[64 * R:64 * R + 64,
                                            base:base + 512].rearrange(
                                                "p (h w) -> p h w", w=W)
                                    if dwi == 0:   # dw=-1
                                        nc.tensor.matmul(
                                            o3[:, :, 1:W], lhsT,
                                            r3[:, :, 0:W - 1],
                                            start=start, stop=stop,
                                            skip_group_check=True,
                                            tile_position=tp)
                                    else:          # dw=+1
                                        nc.tensor.matmul(
                                            o3[:, :, 0:W - 1], lhsT,
                                            r3[:, :, 1:W],
                                            start=start, stop=stop,
                                            skip_group_check=True,
                                            tile_position=tp)
                return pb

            def emit_evac(cc, q, pb_q, om, dve=False):
                # om = psum + bc (bias with folded -corr), f32 -> bf16.
                # Mostly ACT (fastest); a couple of windows' R=1 go to DVE to
                # balance engine load.
                bc = st[cc]["bc"]
                for R in range(2):
                    dst = om[:, R * 8192 + q * 512: R * 8192 + q * 512 + 512]
                    if dve and R == 1:
                        nc.vector.tensor_scalar_add(dst, pb_q[R][:, :],
                                                    bc[:, R:R + 1])
                    else:
                        nc.scalar.activation(dst, pb_q[R][:, :], IDENT,
                                             bias=bc[:, R:R + 1])

            def emit_out(cc, om, hh):
                for Cg in range(2):
                    nc.sync.dma_start(
                        out_v[cc, Cg, hh],
                        om[64 * Cg:64 * Cg + 64, :].rearrange(
                            "p (R hh e) -> p R hh e", hh=2, e=4096)[:, :, hh, :])

            # finer store view for the last chunk's drains (shrinks the tail)
            out_v4 = out_d[:].rearrange("(a R p) (Ch qq e) -> a Ch qq p R e",
                                        a=NCH, R=2, p=64, Ch=2, qq=4, e=2048)

            def emit_out4(cc, om, part):
                for Cg in range(2):
                    nc.sync.dma_start(
                        out_v4[cc, Cg, part],
                        om[64 * Cg:64 * Cg + 64, :].rearrange(
                            "p (R qq e) -> p R qq e", qq=4, e=2048)[:, :, part, :])

            # ---- prologue: chunks 0,1 loads+stats; chunk-0 chain up front.
            # ACT takes slice 0 (lands first) so the slice-3 landing gates
            # only the (faster) DVE share; wscale(0) runs on ACT so it is
            # not queued behind chunk 1's stats on DVE.
            emit_load(0)
            for k in range(3):
                emit_stats_dve(0, k)
            emit_stats_act(0)
            emit_aggr(0)
            emit_load(1)
            for k in range(3):
                emit_stats_dve(1, k)
            emit_stats_act(1)
            emit_mix(0)
            emit_chain(0)
            emit_wscale(0)
            emit_halo(0)
            emit_corr(0)
            emit_bc(0)

            # ---- steady state: per chunk, 8 tap-outer windows of 2 spans.
            # chain(cc+1) is emitted early (w==1) off already-complete stats;
            # stats(cc+2) ride the DVE queue as load slices land.
            for cc in range(NCH):
                om = opool.tile([128, 4 * 4096], bf16, tag="om", name=f"om{cc}")
                for w in range(8):
                    pb = emit_window_mms(cc, w)
                    for q in (2 * w, 2 * w + 1):
                        emit_evac(cc, q, pb[q], om, dve=(w in (3, 6)))
                    if w == 0 and cc + 2 < NCH:
                        emit_load(cc + 2)
                    if w == 1 and cc + 1 < NCH:
                        emit_aggr(cc + 1)
                        emit_mix(cc + 1)
                        emit_chain(cc + 1)
                        emit_wscale(cc + 1)
                        emit_halo(cc + 1)
                    if w in (2, 3, 4) and cc + 2 < NCH:
                        emit_stats_dve(cc + 2, w - 2)
                    if w == 5 and cc + 2 < NCH:
                        emit_stats_act(cc + 2)
                    if cc < NCH - 1:
                        if w == 3:
                            emit_out(cc, om, 0)
                    elif w in (1, 3, 5):
                        emit_out4(cc, om, w // 2)
                if cc + 1 < NCH:
                    emit_corr(cc + 1)
                    emit_bc(cc + 1)
                if cc < NCH - 1:
                    emit_out(cc, om, 1)
                else:
                    emit_out4(cc, om, 3)
    nc.compile()
    return nc


def _pack_inputs(x, dw, pw, biases):
    """Host-side: fuse pw o dw, scatter into block-diag 64x64 lhsT tiles."""
    G = 128
    dwr = dw.reshape(B, G, 4, 4, 3, 3)          # [b, g, m, i, kh, kw]
    pwr = pw.reshape(B, G, 4, 4)                # [b, g, j, m]
    eff = np.einsum('bgjm,bgmikl->bgjikl', pwr, dwr)  # [b, g, j, i, kh, kw]
    # 64x64 block-diag tiles: w_host[b, 64R + 4gl + i, (cc*9+tap)*64 + 4gl + j]
    w_host = np.zeros((B, 128, NCH * NTAP * 64), dtype=np.float32)
    wv = w_host.reshape(B, 2, 16, 4, NCH, NTAP, 16, 4)  # [b,R,gl_k,i,cc,tap,gl_m,j]
    er = eff.reshape(B, NCH, 2, 16, 4, 4, NTAP)         # [b, cc, R, gl, j, i, tap]
    for gl in range(16):
        e = er[:, :, :, gl]                     # [b, cc, R, j, i, tap]
        wv[:, :, gl, :, :, :, gl, :] = e.transpose(0, 2, 4, 1, 5, 3)
    bias_host = np.zeros((B, 128, 8), dtype=np.float32)
    bfull = biases.reshape(B, C)
    p = np.arange(128)
    for cc in range(NCH):
        for R in range(2):
            bias_host[:, :, cc * 2 + R] = bfull[:, cc * 128 + 64 * R + (p % 64)]
    return w_host.astype(ml_dtypes.bfloat16), bias_host


def kernel(x, dw_kernels, pw_kernels, biases):
    from concourse.bass_utils import run_bass_kernel_spmd

    x = np.ascontiguousarray(np.asarray(x, dtype=np.float32))
    dw = np.asarray(dw_kernels, dtype=np.float32)
    pw = np.asarray(pw_kernels, dtype=np.float32)
    bs = np.asarray(biases, dtype=np.float32)

    if "nc" not in _CACHE:
        _CACHE["nc"] = _build_program()
    nc = _CACHE["nc"]

    w_host, bias_host = _pack_inputs(x, dw, pw, bs)
    xb = x.reshape(B, C, HW).astype(ml_dtypes.bfloat16)
    in_maps = [{"x": xb[i],
                "w": w_host[i],
                "bias": bias_host[i]} for i in range(B)]
    res = run_bass_kernel_spmd(nc, in_maps, core_ids=list(range(B)),
                               trace=bool(int(os.environ.get("KTRACE", "0"))))
    _CACHE["last_result"] = res
    out = np.stack([res.results[i]["out"].astype(np.float32).reshape(C, H, W)
                    for i in range(B)])
    return out


# revision 32
# speedup vs baseline: 1.2385x; 1.0402x over previous
"""AdaConv2D (instance-norm -> grouped 3x3 conv -> grouped 1x1 conv -> bias) on 8 TRN2 cores.

v2 strategy (pure data parallel, 1 sample/core, no collectives).  Key change
vs v1 (195.5us): the instance-norm is FOLDED INTO THE WEIGHTS instead of
materialized as a normalize pass over x:

  out = conv(xn) + b,  xn = (x - mean)*rstd
      = conv_{w*rstd}(x_raw) + (b - sum_taps w*rstd*mean)

  - Device computes mean/rstd per channel (DVE bn_stats one pass + tiny ACT
    Ln/Exp chain), scales the chunk's weights by rstd (one ACT op, 576
    elems/partition), and runs the conv directly on RAW bf16 x.
  - H-edges: the 1-row top/bottom halo is filled with the per-channel MEAN
    (not zero): out-of-image taps then contribute w*mean which exactly
    cancels against the folded bias correction.
  - Bias correction corr_j = sum_{i,9 taps} w_s[i,j]*mean_i is computed on
    the PE itself: 36 N=1 matvec accumulations (4 quadrants x 9 taps) into a
    PSUM tile, ~1us/chunk.  bc = bias - corr (gpsimd).
  - W-edges keep the shrunk-AP taps; their (sub-1e-3) bias-corr discrepancy
    is left uncorrected - measured total rel err 1.2e-3 vs budget 2e-2.

  This removes the entire normalize pass (was ~17us/chunk of gpsimd/DVE) and
  the load->stats->chain->normalize critical path that held the first matmul
  until 41us and caused 16us of mid-kernel PE stalls.

  Engine layout per 128-channel chunk (4 chunks/sample):
  - Sync (HWDGE): all DMA. in: 4x1MB slices, two chunks ahead; out: 1MB
    chunks per spatial half (quarters on the last chunk).
  - DVE: bn_stats x32 (one pass over x as slices land) + bn_aggr.
  - ACT: rstd chain (Ln/Exp), mean->bf16, weight scaling, and ALL PSUM
    eviction (activation Identity with per-partition bias AP, psum f32 ->
    bf16 staging; ACT is the fastest eviction engine at ~0.71ns/elem).
  - GpSimd: halo fills (2 broadcast ops) + bc = bias - corr.  Nothing bulk.
  - TensorE: conv as 4 concurrent 64x64 tile_position matmuls (2 channel
    sub-chunk PAIRS x 2 spatial halves), 9 taps accumulated in PSUM.
    TAP-OUTER over windows of 2 spans: per tap one pair of lhsT loads serves
    8 matmuls (2 spans x 4 quadrants), halving exposure to the 64-col
    LDWEIGHTS serialization that bounds v1 (~212ns/tap of weight load vs
    ~107ns of bf16 moving-operand streaming).
  - PSUM: single 8-buf pool; each window holds 4 banks, corr rides the same
    rotation as a full-bank tile once per chunk.
"""
import os
import sys
import numpy as np
import ml_dtypes

if "/opt/trn_rl_repo" not in sys.path:
    sys.path.insert(0, "/opt/trn_rl_repo")

B, C, H, W = 8, 512, 128, 128
HW = H * W            # 16384
NCH = 4               # 128-channel chunks per sample
NTAP = 9
ROWS_PAD = H + 2      # 130 rows of 128 in padded SBUF layout
PADF = ROWS_PAD * W   # 16640 elems per partition
# taps ordered so the first three are dw=0 (full-width writes -> correct PSUM init)
TAPS = [(0, 1), (1, 1), (2, 1), (0, 0), (1, 0), (2, 0), (0, 2), (1, 2), (2, 2)]

_CACHE = {}


def _build_program():
    import concourse.bass as bass
    import concourse.tile as tile
    from concourse import bacc, mybir

    f32 = mybir.dt.float32
    bf16 = mybir.dt.bfloat16
    COPY = mybir.ActivationFunctionType.Copy
    IDENT = mybir.ActivationFunctionType.Identity
    SQRT = mybir.ActivationFunctionType.Sqrt
    EPS = 1e-7
    nc = bacc.Bacc("TRN2", target_bir_lowering=False, debug=False,
                   enable_asserts=False, num_devices=8)

    x_d = nc.dram_tensor("x", [C, HW], bf16, kind="ExternalInput")
    w_d = nc.dram_tensor("w", [128, NCH * NTAP * 64], bf16, kind="ExternalInput")
    b_d = nc.dram_tensor("bias", [128, 8], f32, kind="ExternalInput")
    out_d = nc.dram_tensor("out", [C, HW], bf16, kind="ExternalOutput")

    # store view: [cc, Ch(spatial half), hh(drain half), p, R, e(4096)]
    out_v = out_d[:].rearrange("(a R p) (Ch hh e) -> a Ch hh p R e", a=NCH, R=2,
                               p=64, Ch=2, hh=2, e=4096)

    with tile.TileContext(nc) as tc:
        with (
            tc.tile_pool(name="xpool", bufs=3) as xpool,
            tc.tile_pool(name="wpool", bufs=1) as wpool,
            tc.tile_pool(name="spool", bufs=3) as spool,
            tc.tile_pool(name="opool", bufs=2) as opool,
            tc.tile_pool(name="psum", bufs=8, space=bass.MemorySpace.PSUM) as pspool,
        ):
            # Stage weights/bias through a one-shot copy: the DMA sem then
            # has a single early consumer, so its reuse-reset does not wait
            # for chunk-3-era wscale/bc instructions (measured 78us sync
            # stall blocking all later DMA issues).
            w_dma = wpool.tile([128, NCH * NTAP * 64], bf16)
            nc.sync.dma_start(w_dma[:], w_d[:])
            w_sb = wpool.tile([128, NCH * NTAP * 64], bf16)
            nc.vector.tensor_scalar_mul(w_sb[:], w_dma[:], 1.0)
            bias_dma = wpool.tile([128, 8], f32)
            nc.sync.dma_start(bias_dma[:], b_d[:])
            bias_sb = wpool.tile([128, 8], f32)
            nc.vector.tensor_scalar_mul(bias_sb[:], bias_dma[:], 1.0)
            ones_sb = wpool.tile([128, W], bf16)
            nc.gpsimd.memset(ones_sb[:], 1.0)
            trash0 = wpool.tile([128, 4096], bf16)
            trash1 = wpool.tile([128, 4096], bf16)

            st = {}  # per-chunk small tiles

            def emit_load(cc):
                xt = xpool.tile([128, PADF], bf16, tag="xt", name=f"xt{cc}")
                st[cc] = {"xt": xt}
                for k in range(2):
                    nc.sync.dma_start(xt[:, W + k * 8192: W + (k + 1) * 8192],
                                      x_d[cc * 128:(cc + 1) * 128,
                                          k * 8192:(k + 1) * 8192])

            def emit_stats_dve(cc, k, pos=None):
                # DVE bn_stats over slice k (8 blocks of 512), single pass;
                # pos = 8-block position within the 24-block stats6 tile
                s = st[cc]
                if pos is None:
                    pos = k
                if pos == 0:
                    s["stats6"] = spool.tile([128, 24 * 6], f32, tag="stats",
                                             name=f"st{cc}")
                xt = s["xt"]
                for jj in range(8):
                    j = 8 * k + jj
                    o = (8 * pos + jj) * 6
                    nc.vector.bn_stats(s["stats6"][:, o:o + 6],
                                       xt[:, W + j * 512: W + (j + 1) * 512])

            def emit_stats_act(cc, k=3):
                # ACT: sum + sumsq of slice k via Copy/Square accumulators
                s = st[cc]
                acc = spool.tile([128, 2], f32, tag="acc", name=f"ac{cc}")
                sl = s["xt"][:, W + k * 4096: W + (k + 1) * 4096]
                nc.scalar.activation(trash0[:], sl, COPY,
                                     accum_out=acc[:, 0:1])
                nc.scalar.activation(trash1[:], sl,
                                     mybir.ActivationFunctionType.Square,
                                     accum_out=acc[:, 1:2])
                s["acc"] = acc

            def emit_aggr(cc):
                s = st[cc]
                mv = spool.tile([128, 2], f32, tag="mv", name=f"mv{cc}")
                nc.vector.bn_aggr(mv[:], s["stats6"][:].rearrange(
                    "p (h s) -> p h s", s=6))
                s["mv24"] = mv

            def emit_mix(cc):
                # gpsimd: merge DVE 24-block stats (3/4 of chunk) with the
                # ACT accumulators (1/4): mean, var, all [128,1] f32 ops
                s = st[cc]
                g = nc.gpsimd
                ADD = mybir.AluOpType.add
                MUL = mybir.AluOpType.mult
                mv = s["mv24"]
                acc = s["acc"]
                t0 = spool.tile([128, 2], f32, tag="t0", name=f"t0{cc}")
                g.tensor_scalar_mul(t0[:], acc[:], 1.0 / HW)
                t1 = spool.tile([128, 2], f32, tag="t1", name=f"t1{cc}")
                g.tensor_scalar_mul(t1[:], mv[:], 0.75)  # [.75 m24, .75 v24]
                mean = spool.tile([128, 1], f32, tag="mean", name=f"me{cc}")
                g.tensor_add(mean[:], t1[:, 0:1], t0[:, 0:1])
                m2a = spool.tile([128, 1], f32, tag="m2a", name=f"ma{cc}")
                g.tensor_mul(m2a[:], t1[:, 0:1], mv[:, 0:1])  # .75 m24^2
                e2a = spool.tile([128, 1], f32, tag="e2a", name=f"ea{cc}")
                g.tensor_add(e2a[:], t1[:, 1:2], m2a[:])
                ex2 = spool.tile([128, 1], f32, tag="ex2", name=f"ex{cc}")
                g.tensor_add(ex2[:], e2a[:], t0[:, 1:2])
                m2 = spool.tile([128, 1], f32, tag="m2", name=f"m2{cc}")
                g.tensor_mul(m2[:], mean[:], mean[:])
                var = spool.tile([128, 1], f32, tag="var", name=f"va{cc}")
                g.tensor_sub(var[:], ex2[:], m2[:])
                s["mean"] = mean
                s["var"] = var

            def emit_chain(cc):
                # rstd = rsqrt(var*N/(N-1)) via Newton on DVE: y0 = 2/(1+v)
                # (convergent for all v>0), three steps y *= 1.5 - 0.5*v*y^2.
                # Exact to <1e-5 for v in [0.1, 10]; avoids ACT entirely (no
                # Ln/Exp/Sqrt -> no 1.3us act-table reloads on the evac queue
                # and no cross-queue latency into wscale).
                s = st[cc]
                g = nc.gpsimd
                c = float(HW) / float(HW - 1)
                vc1 = spool.tile([128, 1], f32, tag="vc1", name=f"v1{cc}")
                g.tensor_scalar(vc1[:], s["var"][:], c, 1.0,
                                op0=mybir.AluOpType.mult,
                                op1=mybir.AluOpType.add)      # v*c + 1
                hv = spool.tile([128, 1], f32, tag="hv", name=f"hv{cc}")
                g.tensor_scalar_mul(hv[:], s["var"][:], -0.5 * c)  # -0.5*v*c
                mb = spool.tile([128, 1], bf16, tag="mb", name=f"mb{cc}")
                g.tensor_scalar_mul(mb[:], s["mean"][:], 1.0)
                v = nc.vector
                y = spool.tile([128, 1], f32, tag="y0", name=f"y0{cc}")
                v.reciprocal(y[:], vc1[:])
                yt = spool.tile([128, 4], f32, tag="yt", name=f"yt{cc}")
                v.tensor_scalar_mul(yt[:, 0:1], y[:], 2.0)    # y0 = 2/(1+vc)
                for it in range(3):
                    t = spool.tile([128, 2], f32, tag=f"nt{it}",
                                   name=f"nt{it}_{cc}")
                    v.tensor_mul(t[:, 0:1], yt[:, it:it + 1], yt[:, it:it + 1])
                    v.tensor_scalar(t[:, 1:2], t[:, 0:1], hv[:, 0:1], 1.5,
                                    op0=mybir.AluOpType.mult,
                                    op1=mybir.AluOpType.add)  # 1.5 - .5vcy^2
                    v.tensor_mul(yt[:, it + 1:it + 2], yt[:, it:it + 1],
                                 t[:, 1:2])
                rstd = spool.tile([128, 1], f32, tag="rstd", name=f"rs{cc}")
                v.tensor_scalar_mul(rstd[:], yt[:, 3:4], 1.0)
                s["rstd"] = rstd
                s["mb"] = mb

            def emit_wscale(cc, on_gp=False):
                # scale this chunk's weights by rstd (per input channel row).
                # chunk 0 runs on the idle gpsimd queue so it is not stuck
                # behind chunk 1's bn_stats on DVE in the prologue.
                s = st[cc]
                wsc = spool.tile([128, NTAP * 64], bf16, tag="wsc",
                                 name=f"ws{cc}")
                src = w_sb[:, cc * NTAP * 64:(cc + 1) * NTAP * 64]
                eng = nc.gpsimd if on_gp else nc.vector
                eng.tensor_scalar_mul(wsc[:], src, s["rstd"][:, 0:1])
                s["wsc"] = wsc

            def emit_halo(cc):
                # top/bottom halo row := per-channel mean (bf16) so that
                # out-of-image dh taps cancel against the folded bias corr.
                # scalar operand must be f32; the bf16 output rounds to the
                # same value as the mb tile used by the corr matvec.
                s = st[cc]
                xt = s["xt"]
                mf = s["mean"][:, 0:1]
                nc.gpsimd.tensor_scalar_mul(xt[:, 0:W], ones_sb[:], mf)
                nc.gpsimd.tensor_scalar_mul(xt[:, PADF - W:PADF], ones_sb[:],
                                            mf)

            def emit_corr(cc):
                # corr_j = sum_{taps,i} wsc[i,j]*mean_i via N=1 PE matvecs,
                # accumulated in PSUM (uses a full-bank tile in the rotation)
                s = st[cc]
                wsc = s["wsc"]
                mb = s["mb"]
                cp = pspool.tile([128, 512], f32, tag="pb", name=f"cp{cc}")
                for ti in range(NTAP):
                    start, stop = (ti == 0), (ti == NTAP - 1)
                    for R in range(2):
                        lhsT = wsc[64 * R:64 * R + 64, ti * 64: ti * 64 + 64]
                        for Cg in range(2):
                            nc.tensor.matmul(
                                cp[64 * Cg:64 * Cg + 64, R:R + 1], lhsT,
                                mb[64 * R:64 * R + 64, 0:1],
                                start=start, stop=stop,
                                tile_position=(64 * R, 64 * Cg))
                s["corrp"] = cp

            def emit_bc(cc):
                # bc[:, R] = bias[:, cc*2+R] - corr[:, R] on ACT (gpsimd has
                # no PSUM access): Identity(corr * -1 + bias)
                s = st[cc]
                bc = spool.tile([128, 2], f32, tag="bc", name=f"bc{cc}")
                for R in range(2):
                    nc.scalar.activation(
                        bc[:, R:R + 1], s["corrp"][:, R:R + 1], IDENT,
                        bias=bias_sb[:, cc * 2 + R:cc * 2 + R + 1], scale=-1.0)
                s["bc"] = bc

            def emit_window_mms(cc, w):
                # window = spans (2w, 2w+1); tap-outer so one lhsT pair serves
                # 8 matmuls.  span q covers spatial tiles {q, 16+q} (Cg).
                xt = st[cc]["xt"]
                wsc = st[cc]["wsc"]
                qs = (2 * w, 2 * w + 1)
                pb = {q: [pspool.tile([128, 512], f32, tag="pb",
                                      name=f"pb{cc}_{q}_{R}") for R in range(2)]
                      for q in qs}
                for ti, (dh, dwi) in enumerate(TAPS):
                    start, stop = (ti == 0), (ti == NTAP - 1)
                    tapi = dh * 3 + dwi
                    for R in range(2):
                        lhsT = wsc[64 * R:64 * R + 64,
                                   tapi * 64: tapi * 64 + 64]
                        for q in qs:
                            for Cg in range(2):
                                t = 16 * Cg + q
                                base = (4 * t + dh) * W
                                outp = pb[q][R][64 * Cg:64 * Cg + 64, :]
                                tp = (64 * R, 64 * Cg)
                                if dwi == 1:
                                    nc.tensor.matmul(
                                        outp, lhsT,
                                        xt[64 * R:64 * R + 64, base:base + 512],
                                        start=start, stop=stop,
                                        tile_position=tp)
                                else:
                                    o3 = outp.rearrange("p (h w) -> p h w", w=W)
                                    r3 = xt[64 * R:64 * R + 64,
                                            base:base + 512].rearrange(
                                                "p (h w) -> p h w", w=W)
                                    if dwi == 0:   # dw=-1
                                        nc.tensor.matmul(
                                            o3[:, :, 1:W], lhsT,
                                            r3[:, :, 0:W - 1],
                                            start=start, stop=stop,
                                            skip_group_check=True,
                                            tile_position=tp)
                                    else:          # dw=+1
                                        nc.tensor.matmul(
                                            o3[:, :, 0:W - 1], lhsT,
                                            r3[:, :, 1:W],
                                            start=start, stop=stop,
                                            skip_group_check=True,
                                            tile_position=tp)
                return pb

            def emit_evac(cc, q, pb_q, om, dve=False):
                # om = psum + bc (bias with folded -corr), f32 -> bf16.
                # Mostly ACT (fastest); a couple of windows' R=1 go to DVE to
                # balance engine load.
                bc = st[cc]["bc"]
                for R in range(2):
                    dst = om[:, R * 8192 + q * 512: R * 8192 + q * 512 + 512]
                    if dve and R == 1:
                        nc.vector.tensor_scalar_add(dst, pb_q[R][:, :],
                                                    bc[:, R:R + 1])
                    else:
                        nc.scalar.activation(dst, pb_q[R][:, :], IDENT,
                                             bias=bc[:, R:R + 1])

            def emit_out(cc, om, hh):
                for Cg in range(2):
                    nc.sync.dma_start(
                        out_v[cc, Cg, hh],
                        om[64 * Cg:64 * Cg + 64, :].rearrange(
                            "p (R hh e) -> p R hh e", hh=2, e=4096)[:, :, hh, :])

            # finer store view for the last chunk's drains (shrinks the tail)
            out_v4 = out_d[:].rearrange("(a R p) (Ch qq e) -> a Ch qq p R e",
                                        a=NCH, R=2, p=64, Ch=2, qq=4, e=2048)

            def emit_out4(cc, om, part):
                for Cg in range(2):
                    nc.sync.dma_start(
                        out_v4[cc, Cg, part],
                        om[64 * Cg:64 * Cg + 64, :].rearrange(
                            "p (R qq e) -> p R qq e", qq=4, e=2048)[:, :, part, :])

            # ---- prologue: chunks 0,1 loads+stats; chunk-0 chain up front.
            # ACT takes slice 0 (lands first) so the slice-3 landing gates
            # only the (faster) DVE share; wscale(0) runs on ACT so it is
            # not queued behind chunk 1's stats on DVE.
            emit_load(0)
            for k in range(3):
                emit_stats_dve(0, k)
            emit_stats_act(0)
            emit_aggr(0)
            emit_load(1)
            for k in range(3):
                emit_stats_dve(1, k)
            emit_stats_act(1)
            emit_mix(0)
            emit_chain(0)
            emit_wscale(0)
            emit_halo(0)
            emit_corr(0)
            emit_bc(0)

            # ---- steady state: per chunk, 8 tap-outer windows of 2 spans.
            # chain(cc+1) is emitted early (w==1) off already-complete stats;
            # stats(cc+2) ride the DVE queue as load slices land.
            for cc in range(NCH):
                om = opool.tile([128, 4 * 4096], bf16, tag="om", name=f"om{cc}")
                for w in range(8):
                    pb = emit_window_mms(cc, w)
                    for q in (2 * w, 2 * w + 1):
                        emit_evac(cc, q, pb[q], om, dve=(w in (3, 6)))
                    if w == 0 and cc + 2 < NCH:
                        emit_load(cc + 2)
                    if w == 1 and cc + 1 < NCH:
                        emit_aggr(cc + 1)
                        emit_mix(cc + 1)
                        emit_chain(cc + 1)
                        emit_wscale(cc + 1)
                        emit_halo(cc + 1)
                    if w in (2, 3, 4) and cc + 2 < NCH:
                        emit_stats_dve(cc + 2, w - 2)
                    if w == 5 and cc + 2 < NCH:
                        emit_stats_act(cc + 2)
                    if cc < NCH - 1:
                        if w == 3:
                            emit_out(cc, om, 0)
                    elif w in (1, 3, 5):
                        emit_out4(cc, om, w // 2)
                if cc + 1 < NCH:
                    emit_corr(cc + 1)
                    emit_bc(cc + 1)
                if cc < NCH - 1:
                    emit_out(cc, om, 1)
                else:
                    emit_out4(cc, om, 3)
    nc.compile()
    return nc


def _pack_inputs(x, dw, pw, biases):
    """Host-side: fuse pw o dw, scatter into block-diag 64x64 lhsT tiles."""
    G = 128
    dwr = dw.reshape(B, G, 4, 4, 3, 3)          # [b, g, m, i, kh, kw]
    pwr = pw.reshape(B, G, 4, 4)                # [b, g, j, m]
    eff = np.einsum('bgjm,bgmikl->bgjikl', pwr, dwr)  # [b, g, j, i, kh, kw]
    # 64x64 block-diag tiles: w_host[b, 64R + 4gl + i, (cc*9+tap)*64 + 4gl + j]
    w_host = np.zeros((B, 128, NCH * NTAP * 64), dtype=np.float32)
    wv = w_host.reshape(B, 2, 16, 4, NCH, NTAP, 16, 4)  # [b,R,gl_k,i,cc,tap,gl_m,j]
    er = eff.reshape(B, NCH, 2, 16, 4, 4, NTAP)         # [b, cc, R, gl, j, i, tap]
    for gl in range(16):
        e = er[:, :, :, gl]                     # [b, cc, R, j, i, tap]
        wv[:, :, gl, :, :, :, gl, :] = e.transpose(0, 2, 4, 1, 5, 3)
    bias_host = np.zeros((B, 128, 8), dtype=np.float32)
    bfull = biases.reshape(B, C)
    p = np.arange(128)
    for cc in range(NCH):
        for R in range(2):
            bias_host[:, :, cc * 2 + R] = bfull[:, cc * 128 + 64 * R + (p % 64)]
    return w_host.astype(ml_dtypes.bfloat16), bias_host


def kernel(x, dw_kernels, pw_kernels, biases):
    from concourse.bass_utils import run_bass_kernel_spmd

    x = np.ascontiguousarray(np.asarray(x, dtype=np.float32))
    dw = np.asarray(dw_kernels, dtype=np.float32)
    pw = np.asarray(pw_kernels, dtype=np.float32)
    bs = np.asarray(biases, dtype=np.float32)

    if "nc" not in _CACHE:
        _CACHE["nc"] = _build_program()
    nc = _CACHE["nc"]

    w_host, bias_host = _pack_inputs(x, dw, pw, bs)
    xb = x.reshape(B, C, HW).astype(ml_dtypes.bfloat16)
    in_maps = [{"x": xb[i],
                "w": w_host[i],
                "bias": bias_host[i]} for i in range(B)]
    res = run_bass_kernel_spmd(nc, in_maps, core_ids=list(range(B)),
                               trace=bool(int(os.environ.get("KTRACE", "0"))))
    _CACHE["last_result"] = res
    out = np.stack([res.results[i]["out"].astype(np.float32).reshape(C, H, W)
                    for i in range(B)])
    return out
